# revision 2
# baseline (speedup 1.0000x reference)
"""Cross-attention kernel for 8 Trainium2 NeuronCores.

Problem: nn_CrossAttention (N=2, X=1024, T=4096, D=1024, H=16, hd=64).

Sharding: core c handles batch n = c//4 and head-group hg = c%4
(4 heads = 256 output dims). No cross-core communication.

Host prep per core (numpy, outside HW timing):
  - xT   = previous_output[n].T          (D, X)  bf16
  - ctxT = context[n].T                  (D, T)  bf16
  - w{q,k,v}T = W[256*hg:256*(hg+1)].T   (D, 256) bf16
  - biases sliced per core.

Device (all matmuls contract over the partition dim):
  qT[c,x]  = wqT.T @ xT    (+bq)         kT[c,t] = wkT.T @ ctxT (+bk)
  v[t,c]   = ctxT.T @ wvT  (+bv via K=1 ones matmul)
  S.T[t,x] = kT_h.T @ qT_h   (per head, K=64)
  P.T      = exp(S.T / 8)                 (ScalarE, scale folded in)
  O'.T[65,x] = [V_h | 1].T @ P.T          (ones col gives softmax denom)
  O[x,64]  = transpose(O'.T) rows 0:64 * 1/row64   (PE transpose + DVE)
"""

import numpy as np
import ml_dtypes
from contextlib import ExitStack

import concourse.bass as bass
import concourse.bacc as bacc
import concourse.tile as tile
import concourse.mybir as mybir
from concourse.bass_utils import run_bass_kernel_spmd
from concourse.masks import make_identity

D, H, HD = 1024, 16, 64
N, X, T = 2, 1024, 4096
NCORES = 8
CH = 4            # heads per core
CW = CH * HD      # 256 output cols per core
KT = D // 128     # 8 d-tiles
TT = T // 128     # 32 t-tiles
XTILES = X // 128  # 8 x-tiles
BF16 = mybir.dt.bfloat16
F32 = mybir.dt.float32

_CACHE = {}


def _build_program():
    nc = bacc.Bacc("TRN2", target_bir_lowering=False, debug=False,
                   num_devices=NCORES)

    xt_d = nc.dram_tensor("xt", (D, X), BF16, kind="ExternalInput")
    ctxt_d = nc.dram_tensor("ctxt", (D, T), BF16, kind="ExternalInput")
    wqt_d = nc.dram_tensor("wqt", (D, CW), BF16, kind="ExternalInput")
    wkt_d = nc.dram_tensor("wkt", (D, CW), BF16, kind="ExternalInput")
    wvt_d = nc.dram_tensor("wvt", (D, CW), BF16, kind="ExternalInput")
    bq_d = nc.dram_tensor("bq", (128, 2), F32, kind="ExternalInput")
    bk_d = nc.dram_tensor("bk", (128, 2), F32, kind="ExternalInput")
    bv_d = nc.dram_tensor("bv", (1, CW), BF16, kind="ExternalInput")
    out_d = nc.dram_tensor("out", (X, CW), F32, kind="ExternalOutput")

    with tile.TileContext(nc) as tc, ExitStack() as ctx:
        consts = ctx.enter_context(tc.tile_pool(name="consts", bufs=1))
        pt_pool = ctx.enter_context(tc.tile_pool(name="pt", bufs=3))
        osb_pool = ctx.enter_context(tc.tile_pool(name="osb", bufs=2))
        rc_pool = ctx.enter_context(tc.tile_pool(name="rc", bufs=2))
        pp = ctx.enter_context(
            tc.tile_pool(name="pp", bufs=2, space="PSUM"))
        st_pool = ctx.enter_context(
            tc.tile_pool(name="st", bufs=2, space="PSUM"))
        op_pool = ctx.enter_context(
            tc.tile_pool(name="op", bufs=2, space="PSUM"))

        # ---- resident SBUF tensors ----
        wq_sb = consts.tile([128, KT, CW], BF16)
        wk_sb = consts.tile([128, KT, CW], BF16)
        wv_sb = consts.tile([128, KT, CW], BF16)
        xt_sb = consts.tile([128, KT, X], BF16)
        ctx_sb = consts.tile([128, KT, T], BF16)
        qt_sb = consts.tile([128, 2, X], BF16)
        kt_sb = consts.tile([128, 2, T], BF16)
        vp_sb = consts.tile([128, TT, CH * (HD + 1)], BF16)  # [.., 260]
        out_sb = consts.tile([128, XTILES, CW], F32)
        bq_sb = consts.tile([128, 2], F32)
        bk_sb = consts.tile([128, 2], F32)
        bv_sb = consts.tile([1, CW], BF16)
        ones_sb = consts.tile([1, 128], BF16)
        ident = consts.tile([128, 128], F32)

        # ---- input DMAs ----
        nc.sync.dma_start(wq_sb[:], wqt_d.ap().rearrange("(k p) c -> p k c", p=128))
        nc.sync.dma_start(wk_sb[:], wkt_d.ap().rearrange("(k p) c -> p k c", p=128))
        nc.sync.dma_start(wv_sb[:], wvt_d.ap().rearrange("(k p) c -> p k c", p=128))
        nc.sync.dma_start(xt_sb[:], xt_d.ap().rearrange("(k p) x -> p k x", p=128))
        nc.sync.dma_start(ctx_sb[:], ctxt_d.ap().rearrange("(k p) t -> p k t", p=128))
        nc.sync.dma_start(bq_sb[:], bq_d.ap())
        nc.sync.dma_start(bk_sb[:], bk_d.ap())
        nc.sync.dma_start(bv_sb[:], bv_d.ap())
        nc.gpsimd.memset(ones_sb[:], 1.0)
        make_identity(nc, ident[:])
        # ones columns of V' (one per head): col 65*h + 64 of each t-tile row
        vp_h = vp_sb[:].rearrange("p t (h c) -> p t h c", c=HD + 1)
        nc.gpsimd.memset(vp_h[:, :, :, HD:HD + 1], 1.0)

        # ---- qT projection: [col, x] per col-tile ----
        for ct in range(2):
            for xc in range(2):
                ps = pp.tile([128, 512], F32, tag="proj")
                for dt in range(KT):
                    nc.tensor.matmul(
                        ps[:],
                        wq_sb[:, dt, 128 * ct:128 * (ct + 1)],
                        xt_sb[:, dt, 512 * xc:512 * (xc + 1)],
                        start=(dt == 0), stop=(dt == KT - 1))
                nc.vector.tensor_scalar_add(
                    qt_sb[:, ct, 512 * xc:512 * (xc + 1)], ps[:],
                    bq_sb[:, ct:ct + 1])

        # ---- kT projection: [col, t] ----
        for ct in range(2):
            for tc8 in range(8):
                ps = pp.tile([128, 512], F32, tag="proj")
                for dt in range(KT):
                    nc.tensor.matmul(
                        ps[:],
                        wk_sb[:, dt, 128 * ct:128 * (ct + 1)],
                        ctx_sb[:, dt, 512 * tc8:512 * (tc8 + 1)],
                        start=(dt == 0), stop=(dt == KT - 1))
                nc.vector.tensor_scalar_add(
                    kt_sb[:, ct, 512 * tc8:512 * (tc8 + 1)], ps[:],
                    bk_sb[:, ct:ct + 1])

        # ---- v projection: natural [t, col], + ones bias row ----
        for tt in range(TT):
            ps = pp.tile([128, 512], F32, tag="proj")
            for dt in range(KT):
                nc.tensor.matmul(
                    ps[:, 0:CW],
                    ctx_sb[:, dt, 128 * tt:128 * (tt + 1)],
                    wv_sb[:, dt, :],
                    start=(dt == 0), stop=False)
            nc.tensor.matmul(
                ps[:, 0:CW], ones_sb[0:1, :], bv_sb[0:1, :],
                start=False, stop=True)
            nc.vector.tensor_copy(
                vp_h[:, tt, :, 0:HD],
                ps[:, 0:CW].rearrange("p (h c) -> p h c", c=HD))

        # ---- attention ----
        for hp in range(2):          # head pair = col-tile of qT/kT
            for xc in range(2):      # x chunk of 512
                oacc = [op_pool.tile([65, 512], F32, tag="op", name=f"oacc{h2}")
                        for h2 in range(2)]
                for tt in range(TT):
                    st = st_pool.tile([128, 1024], F32, tag="st")
                    for h2 in range(2):
                        nc.tensor.matmul(
                            st[:, 512 * h2:512 * (h2 + 1)],
                            kt_sb[64 * h2:64 * (h2 + 1), hp,
                                  128 * tt:128 * (tt + 1)],
                            qt_sb[64 * h2:64 * (h2 + 1), hp,
                                  512 * xc:512 * (xc + 1)],
                            start=True, stop=True)
                    pt = pt_pool.tile([128, 1024], BF16)
                    nc.scalar.activation(
                        pt[:], st[:], mybir.ActivationFunctionType.Exp,
                        scale=0.125)
                    for h2 in range(2):
                        h = 2 * hp + h2
                        nc.tensor.matmul(
                            oacc[h2][:],
                            vp_sb[:, tt, 65 * h:65 * (h + 1)],
                            pt[:, 512 * h2:512 * (h2 + 1)],
                            start=(tt == 0), stop=(tt == TT - 1))
                # drain: normalize + transpose to natural layout
                for h2 in range(2):
                    h = 2 * hp + h2
                    ot = osb_pool.tile([65, 512], F32)
                    nc.vector.tensor_copy(ot[:], oacc[h2][:])
                    for s in range(4):
                        tp = pp.tile([128, 65], F32, tag="proj")
                        nc.tensor.transpose(
                            tp[:], ot[:, 128 * s:128 * (s + 1)],
                            ident[0:65, 0:65])
                        rc = rc_pool.tile([128, 1], F32)
                        nc.vector.reciprocal(rc[:], tp[:, 64:65])
                        nc.vector.tensor_scalar_mul(
                            out_sb[:, 4 * xc + s, 64 * h:64 * (h + 1)],
                            tp[:, 0:64], rc[:])

        nc.sync.dma_start(
            out_d.ap().rearrange("(xt p) c -> p xt c", p=128), out_sb[:])

    nc.compile()
    return nc


def get_program():
    if "nc" not in _CACHE:
        _CACHE["nc"] = _build_program()
    return _CACHE["nc"]


def _shard_inputs(previous_output, context, Wq, bq, Wk, bk, Wv, bv):
    bf = ml_dtypes.bfloat16
    xt = [np.ascontiguousarray(previous_output[n].T).astype(bf)
          for n in range(N)]
    ctxt = [np.ascontiguousarray(context[n].T).astype(bf) for n in range(N)]
    in_maps = []
    for c in range(NCORES):
        n, hg = c // CH, c % CH
        sl = slice(CW * hg, CW * (hg + 1))
        in_maps.append({
            "xt": xt[n],
            "ctxt": ctxt[n],
            "wqt": np.ascontiguousarray(Wq[sl].T).astype(bf),
            "wkt": np.ascontiguousarray(Wk[sl].T).astype(bf),
            "wvt": np.ascontiguousarray(Wv[sl].T).astype(bf),
            "bq": np.ascontiguousarray(
                bq[sl].reshape(2, 128).T).astype(np.float32),
            "bk": np.ascontiguousarray(
                bk[sl].reshape(2, 128).T).astype(np.float32),
            "bv": bv[sl].reshape(1, CW).astype(bf),
        })
    return in_maps


LAST_RESULTS = None


def kernel(previous_output, context, Wq, bq, Wk, bk, Wv, bv):
    global LAST_RESULTS
    previous_output = np.asarray(previous_output, dtype=np.float32)
    context = np.asarray(context, dtype=np.float32)
    Wq = np.asarray(Wq, dtype=np.float32)
    Wk = np.asarray(Wk, dtype=np.float32)
    Wv = np.asarray(Wv, dtype=np.float32)
    bq = np.asarray(bq, dtype=np.float32)
    bk = np.asarray(bk, dtype=np.float32)
    bv = np.asarray(bv, dtype=np.float32)

    nc = get_program()
    in_maps = _shard_inputs(previous_output, context, Wq, bq, Wk, bk, Wv, bv)
    res = run_bass_kernel_spmd(nc, in_maps, core_ids=list(range(NCORES)))
    LAST_RESULTS = res

    out = np.empty((N, X, D), dtype=np.float32)
    for c in range(NCORES):
        n, hg = c // CH, c % CH
        out[n, :, CW * hg:CW * (hg + 1)] = res.results[c]["out"]
    return out
